# revision 1
# baseline (speedup 1.0000x reference)
"""Multi-head causal attention (B=4, S=2048, D=1024, H=16) on 8 trn2 cores.

Sharding: tensor-parallel over heads x data-parallel over batch.
core c -> (batch b = c//2, head-group hg = c%2 of 8 heads). Every core runs
an identical SPMD program on different data:
  - QKV projections for its 512 features (8 heads). K kept transposed
    [feat, seq] in SBUF, V kept [seq, feat] with an appended ones column per
    head (softmax denominators come free out of the PV matmul), Q produced
    per 512-query superblock just in time.
  - Causal attention per (head, superblock): S^T = K^T.T @ Q^T per 128-key
    block as two N=256 matmuls (f32r K=64/M=128 is half-rate at N=512), exp
    on ScalarE with no max subtraction (scores are O(5), exp cannot
    overflow), 0/1 mask multiply on diagonal blocks, PV accumulation in
    PSUM with an M=128-padded stationary.
  - Output projection against the head-group's 512-column slice of Wo.
Host sums the two partial outputs per batch (the "all-reduce after W_o"
done at gather time) and folds the Wo @ bv + bo constant.

Two trn2-specific tricks:
  - All matmuls run in float32r (11-bit mantissa, 4x fp32 PE rate); inputs
    are pre-rounded on the host (RNE at bit 12) so DMA feeds matmul tiles
    directly.
  - The PE HAM clock-gate does not count K=64 matmuls as "busy", so a pure
    attention phase runs at 1.2 GHz. The projection and output-projection
    chains (K=128) are therefore interleaved INTO the attention stream,
    which keeps the clock at 2.4 GHz: projections for superblock sc+1 and
    the output projection for sc-1 are emitted between attention batches
    of superblock sc.
"""

import sys

import numpy as np

_BASS_PATH = "/opt/trn_rl_repo"
if _BASS_PATH not in sys.path:
    sys.path.insert(0, _BASS_PATH)

B, S, D, H, DK = 4, 2048, 1024, 16, 64
NCORES = 8
FH = 512  # features per core (8 heads)
HL = 8  # local heads
NSC = 4  # seq superblocks of 512
SQ = 512
NKB = 16  # key blocks of 128
NDM = 8  # d_model chunks of 128

_cache = {}


def _round_f32r(x: np.ndarray) -> np.ndarray:
    """Round fp32 to fp32r (RNE to 11 mantissa bits) - matches TRN2 HW."""
    v = np.ascontiguousarray(x, dtype=np.float32).view(np.uint32)
    lsb = (v >> np.uint32(12)) & np.uint32(1)
    out = ((v + np.uint32(0x7FF) + lsb) >> np.uint32(12)) << np.uint32(12)
    return out.view(np.float32)


def _build():
    import concourse.bacc as bacc
    import concourse.mybir as mybir
    from concourse.tile import TileContext

    f32, f32r = mybir.dt.float32, mybir.dt.float32r
    AF = mybir.ActivationFunctionType

    nc = bacc.Bacc("TRN2", target_bir_lowering=False, debug=False, num_devices=1)

    xq_d = nc.dram_tensor("xq", [D, S], f32r, kind="ExternalInput").ap()
    xk_d = nc.dram_tensor("xk", [D, S], f32r, kind="ExternalInput").ap()
    xv_d = nc.dram_tensor("xv", [D, S], f32r, kind="ExternalInput").ap()
    wq_d = nc.dram_tensor("wq", [D, FH], f32r, kind="ExternalInput").ap()
    wk_d = nc.dram_tensor("wk", [D, FH], f32r, kind="ExternalInput").ap()
    wv_d = nc.dram_tensor("wv", [D, FH], f32r, kind="ExternalInput").ap()
    wo_d = nc.dram_tensor("wo", [FH, D], f32r, kind="ExternalInput").ap()
    # master causal mask [128, 896]: m[k, c] = 1 iff k <= c - 384.
    # mask_j (j = diag block index) = master[:, (3-j)*128 : (3-j)*128+512]
    mask_d = nc.dram_tensor("masks", [128, 896], f32r, kind="ExternalInput").ap()
    bq_d = nc.dram_tensor("bq", [FH], f32, kind="ExternalInput").ap()
    bk_d = nc.dram_tensor("bk", [FH], f32, kind="ExternalInput").ap()
    out_d = nc.dram_tensor("out", [S, D], f32, kind="ExternalOutput").ap()

    with TileContext(nc) as tc:
        with (
            tc.tile_pool(name="res", bufs=1) as res,
            tc.tile_pool(name="st", bufs=1) as st,
            tc.tile_pool(name="psum", bufs=1, space="PSUM") as psp,
            tc.tile_pool(name="dram", bufs=1, space="DRAM") as dpool,
        ):
            kt = [res.tile([128, S], f32r, name=f"kt{i}", tag=f"kt{i}") for i in range(4)]
            # 520 data cols (8 heads x (64 V + ones)) + pad so PV can read a
            # 128-wide stationary slice for head 7 (rows 65+ of the PV output
            # are garbage and ignored)
            vaug = [
                res.tile([128, 584], f32r, name=f"va{k}", tag=f"va{k}")
                for k in range(NKB)
            ]
            master = res.tile([128, 896], f32r, name="master", tag="master")
            nc.sync.dma_start(master[:], mask_d[:])
            bq_t = [res.tile([128, 1], f32, name=f"bq{i}", tag=f"bq{i}") for i in range(4)]
            bk_t = [res.tile([128, 1], f32, name=f"bk{i}", tag=f"bk{i}") for i in range(4)]
            for i in range(4):
                nc.sync.dma_start(
                    bq_t[i][:],
                    bq_d[i * 128 : (i + 1) * 128].rearrange("(p o) -> p o", o=1),
                )
                nc.sync.dma_start(
                    bk_t[i][:],
                    bk_d[i * 128 : (i + 1) * 128].rearrange("(p o) -> p o", o=1),
                )
            ones_t = res.tile([128, HL], f32, name="ones", tag="ones")
            nc.vector.memset(ones_t[:], 1.0)
            wo_sb = []
            for fc in range(4):
                wt = res.tile([128, D], f32r, name=f"wo{fc}", tag=f"wo{fc}")
                nc.sync.dma_start(wt[:], wo_d[fc * 128 : (fc + 1) * 128, :])
                wo_sb.append(wt)
            ctxd = dpool.tile([FH, S], f32r, name="ctxd", tag="ctxd")

            qsf_box = {}  # sc -> [4 q-slice tiles]

            def make_proj_thunks(sc):
                thunks = []
                for pname, x_d, w_d in (
                    ("k", xk_d, wk_d),
                    ("v", xv_d, wv_d),
                    ("q", xq_d, wq_d),
                ):
                    box = {}

                    def load(pname=pname, x_d=x_d, w_d=w_d, box=box):
                        w_sb, xr = [], []
                        for dm in range(NDM):
                            wt = st.tile(
                                [128, FH], f32r, name=f"w{dm}", tag=f"w{dm}", bufs=2
                            )
                            nc.sync.dma_start(wt[:], w_d[dm * 128 : (dm + 1) * 128, :])
                            w_sb.append(wt)
                            xt = st.tile(
                                [128, SQ], f32r, name=f"x{dm}", tag=f"x{dm}", bufs=1
                            )
                            nc.sync.dma_start(
                                xt[:],
                                x_d[dm * 128 : (dm + 1) * 128, sc * SQ : (sc + 1) * SQ],
                            )
                            xr.append(xt)
                        box["w"], box["x"] = w_sb, xr
                        if pname == "q":
                            qsf_box[sc] = [
                                st.tile(
                                    [128, SQ], f32r, name=f"qs{i}", tag=f"qs{i}", bufs=2
                                )
                                for i in range(4)
                            ]

                    for gi in range(4):

                        def group(pname=pname, gi=gi, box=box, sc=sc, load=load):
                            if gi == 0:
                                load()
                            w_sb, xr = box["w"], box["x"]
                            if pname in ("q", "k"):
                                pp = psp.tile(
                                    [128, SQ], f32, name="pp", tag="pp", bufs=2
                                )
                                for dm in range(NDM):
                                    nc.tensor.matmul(
                                        pp[:],
                                        w_sb[dm][:, gi * 128 : (gi + 1) * 128],
                                        xr[dm][:],
                                        start=(dm == 0),
                                        stop=(dm == NDM - 1),
                                    )
                                if pname == "k":
                                    nc.scalar.activation(
                                        kt[gi][:, sc * SQ : (sc + 1) * SQ],
                                        pp[:],
                                        AF.Identity,
                                        bias=bk_t[gi][:],
                                    )
                                else:
                                    nc.scalar.activation(
                                        qsf_box[sc][gi][:],
                                        pp[:],
                                        AF.Identity,
                                        bias=bq_t[gi][:],
                                    )
                            else:  # v
                                kb = sc * 4 + gi
                                pp = psp.tile(
                                    [128, FH], f32, name="pp", tag="pp", bufs=2
                                )
                                for dm in range(NDM):
                                    nc.tensor.matmul(
                                        pp[:],
                                        xr[dm][:, gi * 128 : (gi + 1) * 128],
                                        w_sb[dm][:],
                                        start=(dm == 0),
                                        stop=(dm == NDM - 1),
                                    )
                                va3 = vaug[kb][:, 0 : HL * 65].rearrange(
                                    "p (h e) -> p h e", e=65
                                )
                                pp3 = pp[:].rearrange("p (h e) -> p h e", e=64)
                                nc.vector.tensor_copy(va3[:, :, 0:64], pp3[:])
                                nc.vector.tensor_copy(
                                    va3[:, :, 64:65],
                                    ones_t[:].rearrange("p (h o) -> p h o", o=1),
                                )

                        thunks.append(group)
                return thunks

            def make_attn_batches(h, sb):
                """Return list of batch thunks for one (head, superblock)."""
                ti, po = h // 2, (h % 2) * 64
                nkb = 4 * (sb + 1)
                kbs = list(range(4 * sb, 4 * sb + 4)) + list(range(4 * sb))
                state = {}

                def batch(b0):
                    if b0 == 0:
                        state["cp"] = psp.tile(
                            [128, SQ], f32, name="cp", tag="cp", bufs=2
                        )
                        state["emitted"] = 0
                    cp = state["cp"]
                    group = []
                    for i in range(b0, b0 + 4):
                        kb = kbs[i]
                        sp = psp.tile([128, SQ], f32, name="sp", tag="sp", bufs=4)
                        for n0 in (0, 256):
                            nc.tensor.matmul(
                                sp[:, n0 : n0 + 256],
                                kt[ti][po : po + 64, kb * 128 : (kb + 1) * 128],
                                qsf_box[sb][ti][po : po + 64, n0 : n0 + 256],
                                start=True,
                                stop=True,
                            )
                        es = st.tile([128, SQ], f32r, name="es", tag="es", bufs=5)
                        nc.scalar.activation(es[:], sp[:], AF.Exp)
                        if kb >= sb * 4:
                            j = kb - sb * 4
                            es2 = st.tile(
                                [128, SQ], f32r, name="es2", tag="es2", bufs=5
                            )
                            nc.vector.tensor_mul(
                                es2[:],
                                es[:],
                                master[:, (3 - j) * 128 : (3 - j) * 128 + 512],
                            )
                            es = es2
                        group.append((kb, es))
                    for off in reversed(range(4)):
                        kb, es = group[off]
                        nc.tensor.matmul(
                            cp[:],
                            vaug[kb][:, h * 65 : h * 65 + 128],
                            es[:],
                            start=(state["emitted"] == 0),
                            stop=(state["emitted"] == nkb - 1),
                        )
                        state["emitted"] += 1
                    if b0 + 4 >= nkb:
                        # normalize and spill ctx^T slice to DRAM
                        d1 = st.tile([1, SQ], f32, name="d1", tag="d1", bufs=2)
                        nc.scalar.copy(d1[:], cp[64:65, :])
                        rb = st.tile([64, SQ], f32, name="rb", tag="rb", bufs=2)
                        nc.gpsimd.partition_broadcast(rb[:], d1[:])
                        rc = st.tile([64, SQ], f32, name="rc", tag="rc", bufs=2)
                        nc.vector.reciprocal_approx_fast(rc[:], rb[:])
                        nrm = st.tile([64, SQ], f32r, name="nrm", tag="nrm", bufs=2)
                        nc.vector.tensor_mul(nrm[:], cp[0:64, :], rc[:])
                        nc.sync.dma_start(
                            ctxd[h * 64 : (h + 1) * 64, sb * SQ : (sb + 1) * SQ],
                            nrm[:],
                        )

                return [
                    (lambda b0=b0: batch(b0)) for b0 in range(0, nkb, 4)
                ]

            def make_o_thunks(sb):
                thunks = []
                box = {}

                def load(sb=sb, box=box):
                    cfc = []
                    for fc in range(4):
                        ct = st.tile(
                            [128, SQ], f32r, name=f"cf{fc}", tag=f"cf{fc}", bufs=1
                        )
                        nc.sync.dma_start(
                            ct[:],
                            ctxd[fc * 128 : (fc + 1) * 128, sb * SQ : (sb + 1) * SQ],
                        )
                        cfc.append(ct)
                    box["c"] = cfc

                for qb in range(4):
                    for n2 in range(2):

                        def group(qb=qb, n2=n2, sb=sb, box=box):
                            if qb == 0 and n2 == 0:
                                load()
                            cfc = box["c"]
                            pp = psp.tile([128, SQ], f32, name="pp", tag="pp", bufs=2)
                            for fc in range(4):
                                nc.tensor.matmul(
                                    pp[:],
                                    cfc[fc][:, qb * 128 : (qb + 1) * 128],
                                    wo_sb[fc][:, n2 * SQ : (n2 + 1) * SQ],
                                    start=(fc == 0),
                                    stop=(fc == 3),
                                )
                            ob = st.tile([128, SQ], f32, name="ob", tag="ob", bufs=2)
                            nc.vector.tensor_copy(ob[:], pp[:])
                            nc.sync.dma_start(
                                out_d[
                                    sb * SQ + qb * 128 : sb * SQ + (qb + 1) * 128,
                                    n2 * SQ : (n2 + 1) * SQ,
                                ],
                                ob[:],
                            )

                        thunks.append(group)
                return thunks

            dummy_state = {"n": 0}

            def make_dummy_thunks(n):
                thunks = []
                for _ in range(n):

                    def g():
                        dp = psp.tile([128, SQ], f32, name="dp", tag="pp", bufs=2)
                        for t in range(4):
                            nc.tensor.matmul(
                                dp[:],
                                wo_sb[t][:, 0:128],
                                wo_sb[(t + 1) % 4][:, 0:SQ],
                                start=(t == 0),
                                stop=(t == 3),
                            )

                    thunks.append(g)
                return thunks

            # ---- emission schedule ----
            for t in make_proj_thunks(0):
                t()
            for sb in range(NSC):
                batches = []
                for h in range(HL):
                    batches += make_attn_batches(h, sb)
                warm = []
                if sb < NSC - 1:
                    warm += make_proj_thunks(sb + 1)
                if sb >= 1:
                    warm += make_o_thunks(sb - 1)
                # pad the warm stream so ~1 in 4 PE chains is K=128 (keeps
                # the HAM clock-gate at full rate through the attention tail)
                want = (len(batches) - len(warm)) // 3
                if want > 0:
                    warm += make_dummy_thunks(want)
                    # re-spread: alternate real and dummy warm items
                    real = warm[: len(warm) - want]
                    dum = warm[len(warm) - want :]
                    mixed = []
                    di = 0
                    for i, w in enumerate(real):
                        mixed.append(w)
                        while di < len(dum) and (di + 1) * len(real) <= (i + 1) * len(dum):
                            mixed.append(dum[di])
                            di += 1
                    mixed += dum[di:]
                    warm = mixed
                nb, nw = len(batches), len(warm)
                wi = 0
                for bi, bt in enumerate(batches):
                    bt()
                    while wi < nw and (wi + 1) * nb <= (bi + 1) * nw:
                        warm[wi]()
                        wi += 1
                while wi < nw:
                    warm[wi]()
                    wi += 1
            for t in make_o_thunks(NSC - 1):
                t()

    nc.compile()
    return nc


def kernel(
    q,
    k,
    v,
    mask=None,
    Wq=None,
    bq=None,
    Wk=None,
    bk=None,
    Wv=None,
    bv=None,
    Wo=None,
    bo=None,
    **_unused,
):
    from concourse.bass_utils import run_bass_kernel_spmd

    if "nc" not in _cache:
        _cache["nc"] = _build()
    nc = _cache["nc"]

    q = np.asarray(q, np.float32)
    k = np.asarray(k, np.float32)
    v = np.asarray(v, np.float32)
    Wq = np.asarray(Wq, np.float32)
    Wk = np.asarray(Wk, np.float32)
    Wv = np.asarray(Wv, np.float32)
    Wo = np.asarray(Wo, np.float32)
    bq = np.zeros(D, np.float32) if bq is None else np.asarray(bq, np.float32)
    bk = np.zeros(D, np.float32) if bk is None else np.asarray(bk, np.float32)
    bv = np.zeros(D, np.float32) if bv is None else np.asarray(bv, np.float32)
    bo = np.zeros(D, np.float32) if bo is None else np.asarray(bo, np.float32)

    qr, kr, vr = _round_f32r(q), _round_f32r(k), _round_f32r(v)
    Wqr, Wkr, Wvr, Wor = map(_round_f32r, (Wq, Wk, Wv, Wo))

    # master causal mask: m[kk, c] = 1 iff kk <= c - 384
    kk = np.arange(128)[:, None]
    cc = np.arange(896)[None, :]
    masks = (kk <= cc - 384).astype(np.float32)

    xT = {}
    for b in range(B):
        xT[("q", b)] = np.ascontiguousarray(qr[b].T)
        xT[("k", b)] = np.ascontiguousarray(kr[b].T)
        xT[("v", b)] = np.ascontiguousarray(vr[b].T)
    wqs, wks, wvs, wos, bqs, bks = {}, {}, {}, {}, {}, {}
    for hg in range(2):
        sl = slice(hg * FH, (hg + 1) * FH)
        wqs[hg] = np.ascontiguousarray(Wqr[sl, :].T) * np.float32(0.125)
        wks[hg] = np.ascontiguousarray(Wkr[sl, :].T)
        wvs[hg] = np.ascontiguousarray(Wvr[sl, :].T)
        wos[hg] = np.ascontiguousarray(Wor[:, sl].T)
        bqs[hg] = np.ascontiguousarray(bq[sl]) * np.float32(0.125)
        bks[hg] = np.ascontiguousarray(bk[sl])

    in_maps = []
    for c in range(NCORES):
        b, hg = c // 2, c % 2
        in_maps.append(
            {
                "xq": xT[("q", b)],
                "xk": xT[("k", b)],
                "xv": xT[("v", b)],
                "wq": wqs[hg],
                "wk": wks[hg],
                "wv": wvs[hg],
                "wo": wos[hg],
                "masks": masks,
                "bq": bqs[hg],
                "bk": bks[hg],
            }
        )

    res = run_bass_kernel_spmd(nc, in_maps, list(range(NCORES)))
    out = np.empty((B, S, D), np.float32)
    for b in range(B):
        out[b] = res.results[2 * b]["out"] + res.results[2 * b + 1]["out"]
    const = Wo @ bv + bo  # bv/bo contribution (folds exactly through softmax)
    if np.any(const):
        out += const[None, None, :]
    return out



# revision 3
# speedup vs baseline: 1.1129x; 1.1129x over previous
"""Multi-head causal attention (B=4, S=2048, D=1024, H=16) on 8 trn2 cores.

Sharding: tensor-parallel over heads x data-parallel over batch.
core c -> (batch b = c//2, head-group hg = c%2 of 8 heads). Every core runs
an identical SPMD program on different data:
  - QKV projections for its 512 features (8 heads). K kept transposed
    [feat, seq] in SBUF, V kept [seq, feat] with an appended ones column per
    head (softmax denominators come free out of the PV matmul), Q produced
    per 512-query superblock just in time.
  - Causal attention per (head, superblock): S^T = K^T.T @ Q^T per 128-key
    block as a single N<=512 bf16 matmul, trimmed to the causally-visible
    query range for diagonal blocks; exp on ScalarE with no max subtraction
    (scores are O(5), exp cannot overflow); in-place [128,128] triangular
    mask multiply on the partial chunk of each diagonal block; PV
    accumulation in PSUM (full-width first block, trimmed rest).
  - ctx^T written straight into persistent SBUF tiles (even heads by DVE
    output placement, odd heads via a small SBUF->SBUF partition-shift DMA)
    -- no DRAM round-trip. Output projection reads those tiles directly.
Host sums the two partial outputs per batch (the "all-reduce after W_o"
done at gather time) and folds the Wo @ bv + bo constant.

All matmuls run in bf16 (measured 216 ns per N=512 matmul vs 227+ for
f32r, LDWEIGHTS fully hidden, no K=64 penalties); PSUM accumulation stays
fp32, so only input rounding (~0.4%) is lost, well inside the 2e-2 gate.
"""

import sys

import numpy as np

_BASS_PATH = "/opt/trn_rl_repo"
if _BASS_PATH not in sys.path:
    sys.path.insert(0, _BASS_PATH)

B, S, D, H, DK = 4, 2048, 1024, 16, 64
NCORES = 8
FH = 512  # features per core (8 heads)
HL = 8  # local heads
NSC = 4  # seq superblocks of 512
SQ = 512
NKB = 16  # key blocks of 128
NDM = 8  # d_model chunks of 128

_cache = {}


def _build():
    import concourse.bacc as bacc
    import concourse.mybir as mybir
    from concourse.tile import TileContext

    f32, bf16 = mybir.dt.float32, mybir.dt.bfloat16
    AF = mybir.ActivationFunctionType

    nc = bacc.Bacc("TRN2", target_bir_lowering=False, debug=False, num_devices=1)

    xq_d = nc.dram_tensor("xq", [D, S], bf16, kind="ExternalInput").ap()
    xk_d = nc.dram_tensor("xk", [D, S], bf16, kind="ExternalInput").ap()
    xv_d = nc.dram_tensor("xv", [D, S], bf16, kind="ExternalInput").ap()
    wq_d = nc.dram_tensor("wq", [D, FH], bf16, kind="ExternalInput").ap()
    wk_d = nc.dram_tensor("wk", [D, FH], bf16, kind="ExternalInput").ap()
    wv_d = nc.dram_tensor("wv", [D, FH], bf16, kind="ExternalInput").ap()
    wo_d = nc.dram_tensor("wo", [FH, D], bf16, kind="ExternalInput").ap()
    # triangular causal mask [128, 128]: tri[k, c] = 1 iff k <= c
    tri_d = nc.dram_tensor("tri", [128, 128], bf16, kind="ExternalInput").ap()
    bq_d = nc.dram_tensor("bq", [FH], f32, kind="ExternalInput").ap()
    bk_d = nc.dram_tensor("bk", [FH], f32, kind="ExternalInput").ap()
    out_d = nc.dram_tensor("out", [S, D], bf16, kind="ExternalOutput").ap()

    with TileContext(nc) as tc:
        with (
            tc.tile_pool(name="res", bufs=1) as res,
            tc.tile_pool(name="st", bufs=1) as st,
            tc.tile_pool(name="psum", bufs=1, space="PSUM") as psp,
        ):
            kt = [res.tile([128, S], bf16, name=f"kt{i}", tag=f"kt{i}") for i in range(4)]
            # 520 data cols (8 heads x (64 V + ones)) + pad so PV can read a
            # 128-wide stationary slice for head 7 (rows 65+ of the PV output
            # are garbage and ignored)
            vaug = [
                res.tile([128, 584], bf16, name=f"va{k}", tag=f"va{k}")
                for k in range(NKB)
            ]
            # persistent ctx^T tiles: ctx_sb[sb][fc] is [128, 512]
            ctx_sb = [
                [
                    res.tile([128, SQ], bf16, name=f"cx{sb}{fc}", tag=f"cx{sb}{fc}")
                    for fc in range(4)
                ]
                for sb in range(NSC)
            ]
            tri = res.tile([128, 128], bf16, name="tri", tag="tri")
            nc.sync.dma_start(tri[:], tri_d[:])
            bq_t = [res.tile([128, 1], f32, name=f"bq{i}", tag=f"bq{i}") for i in range(4)]
            bk_t = [res.tile([128, 1], f32, name=f"bk{i}", tag=f"bk{i}") for i in range(4)]
            for i in range(4):
                nc.sync.dma_start(
                    bq_t[i][:],
                    bq_d[i * 128 : (i + 1) * 128].rearrange("(p o) -> p o", o=1),
                )
                nc.sync.dma_start(
                    bk_t[i][:],
                    bk_d[i * 128 : (i + 1) * 128].rearrange("(p o) -> p o", o=1),
                )
            onesf = res.tile([128, HL], f32, name="onesf", tag="onesf")
            nc.vector.memset(onesf[:], 1.0)
            ones_t = res.tile([128, HL], bf16, name="ones", tag="ones")
            nc.vector.tensor_copy(ones_t[:], onesf[:])
            wo_sb = []
            for fc in range(4):
                wt = res.tile([128, D], bf16, name=f"wo{fc}", tag=f"wo{fc}")
                nc.sync.dma_start(wt[:], wo_d[fc * 128 : (fc + 1) * 128, :])
                wo_sb.append(wt)

            qsf_box = {}  # sc -> [4 q-slice tiles]

            def make_proj_thunks(sc):
                thunks = []
                for pname, x_d, w_d in (
                    ("k", xk_d, wk_d),
                    ("v", xv_d, wv_d),
                    ("q", xq_d, wq_d),
                ):
                    box = {}

                    def load(pname=pname, x_d=x_d, w_d=w_d, box=box):
                        w_sb, xr = [], []
                        for dm in range(NDM):
                            wt = st.tile(
                                [128, FH], bf16, name=f"w{dm}", tag=f"w{dm}", bufs=2
                            )
                            nc.sync.dma_start(wt[:], w_d[dm * 128 : (dm + 1) * 128, :])
                            w_sb.append(wt)
                            xt = st.tile(
                                [128, SQ], bf16, name=f"x{dm}", tag=f"x{dm}", bufs=1
                            )
                            nc.sync.dma_start(
                                xt[:],
                                x_d[dm * 128 : (dm + 1) * 128, sc * SQ : (sc + 1) * SQ],
                            )
                            xr.append(xt)
                        box["w"], box["x"] = w_sb, xr
                        if pname == "q":
                            qsf_box[sc] = [
                                st.tile(
                                    [128, SQ], bf16, name=f"qs{i}", tag=f"qs{i}", bufs=2
                                )
                                for i in range(4)
                            ]

                    for gi in range(4):

                        def group(pname=pname, gi=gi, box=box, sc=sc, load=load):
                            if gi == 0:
                                load()
                            w_sb, xr = box["w"], box["x"]
                            if pname in ("q", "k"):
                                pp = psp.tile(
                                    [128, SQ], f32, name="pp", tag="pp", bufs=2
                                )
                                for dm in range(NDM):
                                    nc.tensor.matmul(
                                        pp[:],
                                        w_sb[dm][:, gi * 128 : (gi + 1) * 128],
                                        xr[dm][:],
                                        start=(dm == 0),
                                        stop=(dm == NDM - 1),
                                    )
                                if pname == "k":
                                    nc.scalar.activation(
                                        kt[gi][:, sc * SQ : (sc + 1) * SQ],
                                        pp[:],
                                        AF.Identity,
                                        bias=bk_t[gi][:],
                                    )
                                else:
                                    nc.scalar.activation(
                                        qsf_box[sc][gi][:],
                                        pp[:],
                                        AF.Identity,
                                        bias=bq_t[gi][:],
                                    )
                            else:  # v
                                kb = sc * 4 + gi
                                pp = psp.tile(
                                    [128, FH], f32, name="pp", tag="pp", bufs=2
                                )
                                for dm in range(NDM):
                                    nc.tensor.matmul(
                                        pp[:],
                                        xr[dm][:, gi * 128 : (gi + 1) * 128],
                                        w_sb[dm][:],
                                        start=(dm == 0),
                                        stop=(dm == NDM - 1),
                                    )
                                va3 = vaug[kb][:, 0 : HL * 65].rearrange(
                                    "p (h e) -> p h e", e=65
                                )
                                pp3 = pp[:].rearrange("p (h e) -> p h e", e=64)
                                nc.vector.tensor_copy(va3[:, :, 0:64], pp3[:])
                                nc.vector.tensor_copy(
                                    va3[:, :, 64:65],
                                    ones_t[:].rearrange("p (h o) -> p h o", o=1),
                                )

                        thunks.append(group)
                return thunks

            def make_attn_batches(h, sb):
                """Return list of batch thunks for one (head, superblock).

                kb order: diagonal blocks first [d0, d1, d2, d3] (d0 emitted
                with full query width and start=True; d1..d3 trimmed to their
                causally visible query range), then off-diagonal blocks.
                """
                ti, po = h // 2, (h % 2) * 64
                nkb = 4 * (sb + 1)
                kbs = list(range(4 * sb, 4 * sb + 4)) + list(range(4 * sb))
                state = {}

                def batch(b0):
                    if b0 == 0:
                        state["cp"] = psp.tile(
                            [128, SQ], f32, name="cp", tag="cp", bufs=2
                        )
                        state["emitted"] = 0
                    cp = state["cp"]
                    group = []
                    for i in range(b0, b0 + 4):
                        kb = kbs[i]
                        dj = kb - sb * 4  # diagonal index (0..3) or negative
                        q0 = dj * 128 if 0 <= dj else 0  # visible query start
                        sp = psp.tile([128, SQ], f32, name="sp", tag="sp", bufs=4)
                        nc.tensor.matmul(
                            sp[:, q0:SQ],
                            kt[ti][po : po + 64, kb * 128 : (kb + 1) * 128],
                            qsf_box[sb][ti][po : po + 64, q0:SQ],
                            start=True,
                            stop=True,
                        )
                        es = st.tile([128, SQ], bf16, name="es", tag="es", bufs=5)
                        nc.scalar.activation(es[:, q0:SQ], sp[:, q0:SQ], AF.Exp)
                        if 0 <= dj:
                            # in-place triangular mask on the partial chunk
                            nc.vector.tensor_mul(
                                es[:, q0 : q0 + 128],
                                es[:, q0 : q0 + 128],
                                tri[:],
                            )
                        group.append((kb, q0, es))
                    for kb, q0, es in group:
                        nc.tensor.matmul(
                            cp[:, q0:SQ],
                            vaug[kb][:, h * 65 : h * 65 + 128],
                            es[:, q0:SQ],
                            start=(state["emitted"] == 0),
                            stop=(state["emitted"] == nkb - 1),
                        )
                        state["emitted"] += 1
                    if b0 + 4 >= nkb:
                        # normalize and write ctx^T slice into SBUF ctx tiles
                        fc, prow = h // 2, (h % 2) * 64
                        d1 = st.tile([1, SQ], f32, name="d1", tag="d1", bufs=2)
                        nc.vector.tensor_copy(d1[:], cp[64:65, :])
                        rb = st.tile([64, SQ], f32, name="rb", tag="rb", bufs=2)
                        nc.gpsimd.partition_broadcast(rb[:], d1[:])
                        rc = st.tile([64, SQ], f32, name="rc", tag="rc", bufs=2)
                        nc.vector.reciprocal_approx_fast(rc[:], rb[:])
                        if prow == 0:
                            nc.vector.tensor_mul(
                                ctx_sb[sb][fc][0:64, :], cp[0:64, :], rc[:]
                            )
                        else:
                            nrm = st.tile(
                                [64, SQ], bf16, name="nrm", tag="nrm", bufs=2
                            )
                            nc.vector.tensor_mul(nrm[:], cp[0:64, :], rc[:])
                            # partition shift 0-63 -> 64-127 via SBUF->SBUF DMA
                            nc.sync.dma_start(ctx_sb[sb][fc][64:128, :], nrm[:])

                return [
                    (lambda b0=b0: batch(b0)) for b0 in range(0, nkb, 4)
                ]

            def make_o_thunks(sb):
                thunks = []
                for qb in range(4):
                    for n2 in range(2):

                        def group(qb=qb, n2=n2, sb=sb):
                            pp = psp.tile([128, SQ], f32, name="pp", tag="pp", bufs=2)
                            for fc in range(4):
                                nc.tensor.matmul(
                                    pp[:],
                                    ctx_sb[sb][fc][:, qb * 128 : (qb + 1) * 128],
                                    wo_sb[fc][:, n2 * SQ : (n2 + 1) * SQ],
                                    start=(fc == 0),
                                    stop=(fc == 3),
                                )
                            ob = st.tile([128, SQ], bf16, name="ob", tag="ob", bufs=2)
                            nc.vector.tensor_copy(ob[:], pp[:])
                            nc.sync.dma_start(
                                out_d[
                                    sb * SQ + qb * 128 : sb * SQ + (qb + 1) * 128,
                                    n2 * SQ : (n2 + 1) * SQ,
                                ],
                                ob[:],
                            )

                        thunks.append(group)
                return thunks

            # ---- emission schedule ----
            for t in make_proj_thunks(0):
                t()
            for sb in range(NSC):
                batches = []
                for h in range(HL):
                    batches += make_attn_batches(h, sb)
                warm = []
                if sb < NSC - 1:
                    warm += make_proj_thunks(sb + 1)
                if sb >= 1:
                    warm += make_o_thunks(sb - 1)
                nb, nw = len(batches), len(warm)
                wi = 0
                for bi, bt in enumerate(batches):
                    bt()
                    while wi < nw and (wi + 1) * nb <= (bi + 1) * nw:
                        warm[wi]()
                        wi += 1
                while wi < nw:
                    warm[wi]()
                    wi += 1
            for t in make_o_thunks(NSC - 1):
                t()

    nc.compile()
    return nc


def kernel(
    q,
    k,
    v,
    mask=None,
    Wq=None,
    bq=None,
    Wk=None,
    bk=None,
    Wv=None,
    bv=None,
    Wo=None,
    bo=None,
    **_unused,
):
    import ml_dtypes
    from concourse.bass_utils import run_bass_kernel_spmd

    if "nc" not in _cache:
        _cache["nc"] = _build()
    nc = _cache["nc"]

    bf = ml_dtypes.bfloat16
    q = np.asarray(q, np.float32)
    k = np.asarray(k, np.float32)
    v = np.asarray(v, np.float32)
    Wq = np.asarray(Wq, np.float32)
    Wk = np.asarray(Wk, np.float32)
    Wv = np.asarray(Wv, np.float32)
    Wo = np.asarray(Wo, np.float32)
    bq = np.zeros(D, np.float32) if bq is None else np.asarray(bq, np.float32)
    bk = np.zeros(D, np.float32) if bk is None else np.asarray(bk, np.float32)
    bv = np.zeros(D, np.float32) if bv is None else np.asarray(bv, np.float32)
    bo = np.zeros(D, np.float32) if bo is None else np.asarray(bo, np.float32)

    # triangular causal mask [128, 128]: tri[kk, c] = 1 iff kk <= c
    kk = np.arange(128)[:, None]
    cc = np.arange(128)[None, :]
    tri = (kk <= cc).astype(bf)

    xT = {}
    for b in range(B):
        xT[("q", b)] = np.ascontiguousarray(q[b].T).astype(bf)
        xT[("k", b)] = np.ascontiguousarray(k[b].T).astype(bf)
        xT[("v", b)] = np.ascontiguousarray(v[b].T).astype(bf)
    wqs, wks, wvs, wos, bqs, bks = {}, {}, {}, {}, {}, {}
    for hg in range(2):
        sl = slice(hg * FH, (hg + 1) * FH)
        wqs[hg] = np.ascontiguousarray(Wq[sl, :].T * np.float32(0.125)).astype(bf)
        wks[hg] = np.ascontiguousarray(Wk[sl, :].T).astype(bf)
        wvs[hg] = np.ascontiguousarray(Wv[sl, :].T).astype(bf)
        wos[hg] = np.ascontiguousarray(Wo[:, sl].T).astype(bf)
        bqs[hg] = np.ascontiguousarray(bq[sl]) * np.float32(0.125)
        bks[hg] = np.ascontiguousarray(bk[sl])

    in_maps = []
    for c in range(NCORES):
        b, hg = c // 2, c % 2
        in_maps.append(
            {
                "xq": xT[("q", b)],
                "xk": xT[("k", b)],
                "xv": xT[("v", b)],
                "wq": wqs[hg],
                "wk": wks[hg],
                "wv": wvs[hg],
                "wo": wos[hg],
                "tri": tri,
                "bq": bqs[hg],
                "bk": bks[hg],
            }
        )

    res = run_bass_kernel_spmd(nc, in_maps, list(range(NCORES)))
    out = np.empty((B, S, D), np.float32)
    for b in range(B):
        out[b] = np.asarray(res.results[2 * b]["out"], np.float32) + np.asarray(
            res.results[2 * b + 1]["out"], np.float32
        )
    const = Wo @ bv + bo  # bv/bo contribution (folds exactly through softmax)
    if np.any(const):
        out += const[None, None, :]
    return out


# revision 5
# speedup vs baseline: 1.2344x; 1.1092x over previous
"""Multi-head causal attention (B=4, S=2048, D=1024, H=16) on 8 trn2 cores.

Sharding: tensor-parallel over heads x data-parallel over batch.
core c -> (batch b = c//2, head-group hg = c%2 of 8 heads). Every core runs
an identical SPMD program on different data:
  - QKV projections for its 512 features (8 heads). K kept transposed
    [feat, seq] in SBUF, V kept [seq, feat] with an appended ones column per
    head (softmax denominators come free out of the PV matmul), Q produced
    per 512-query superblock just in time. Weights live in SBUF for the
    whole kernel (one 1 MB DMA each); x arrives as one prepacked DMA per
    (projection, superblock).
  - Causal attention per (head, superblock): S^T = K^T.T @ Q^T per 128-key
    block as a single N<=512 bf16 matmul, trimmed to the causally-visible
    query range for diagonal blocks; exp on ScalarE (scores are O(5), exp
    cannot overflow, so no max subtraction); in-place [128,128] triangular
    mask multiply on the partial chunk of diagonal blocks; PV accumulation
    in PSUM (full-width first block, trimmed rest). Score and PV batches
    are software-pipelined (PV of batch b is emitted after the scores of
    batch b+1) so the TensorE never waits on the ScalarE exp stream.
  - ctx^T written straight into persistent SBUF tiles (even heads by DVE
    output placement, odd heads via a small SBUF->SBUF partition-shift DMA)
    -- no DRAM round-trip. Output projection reads those tiles directly,
    and is scheduled late (sb2/sb3) to fill the TensorE while the exp-heavy
    attention tail runs on ScalarE.
Host sums the two partial outputs per batch (the "all-reduce after W_o"
done at gather time) and folds the Wo @ bv + bo constant.

All matmuls run in bf16 (measured 216 ns per N=512 matmul vs 227+ for
f32r, LDWEIGHTS fully hidden, no K=64 penalties); PSUM accumulation stays
fp32, so only input rounding (~0.4%) is lost, well inside the 2e-2 gate.
"""

import sys

import numpy as np

_BASS_PATH = "/opt/trn_rl_repo"
if _BASS_PATH not in sys.path:
    sys.path.insert(0, _BASS_PATH)

B, S, D, H, DK = 4, 2048, 1024, 16, 64
NCORES = 8
FH = 512  # features per core (8 heads)
HL = 8  # local heads
NSC = 4  # seq superblocks of 512
SQ = 512
NKB = 16  # key blocks of 128
NDM = 8  # d_model chunks of 128

_cache = {}


def _build():
    import concourse.bacc as bacc
    import concourse.mybir as mybir
    from concourse.tile import TileContext

    f32, bf16 = mybir.dt.float32, mybir.dt.bfloat16
    AF = mybir.ActivationFunctionType

    nc = bacc.Bacc("TRN2", target_bir_lowering=False, debug=False, num_devices=1)

    # x prepacked as [sc, p, dm*SQ]; w prepacked as [p, dm*FH]
    xq_d = nc.dram_tensor("xq", [NSC, 128, NDM * SQ], bf16, kind="ExternalInput").ap()
    xk_d = nc.dram_tensor("xk", [NSC, 128, NDM * SQ], bf16, kind="ExternalInput").ap()
    xv_d = nc.dram_tensor("xv", [NSC, 128, NDM * SQ], bf16, kind="ExternalInput").ap()
    wq_d = nc.dram_tensor("wq", [128, NDM * FH], bf16, kind="ExternalInput").ap()
    wk_d = nc.dram_tensor("wk", [128, NDM * FH], bf16, kind="ExternalInput").ap()
    wv_d = nc.dram_tensor("wv", [128, NDM * FH], bf16, kind="ExternalInput").ap()
    wo_d = nc.dram_tensor("wo", [128, 4 * D], bf16, kind="ExternalInput").ap()
    # triangular causal mask [128, 128]: tri[k, c] = 1 iff k <= c
    tri_d = nc.dram_tensor("tri", [128, 128], bf16, kind="ExternalInput").ap()
    bq_d = nc.dram_tensor("bq", [FH], f32, kind="ExternalInput").ap()
    bk_d = nc.dram_tensor("bk", [FH], f32, kind="ExternalInput").ap()
    out_d = nc.dram_tensor("out", [S, D], bf16, kind="ExternalOutput").ap()

    with TileContext(nc) as tc:
        with (
            tc.tile_pool(name="res", bufs=1) as res,
            tc.tile_pool(name="st", bufs=1) as st,
            tc.tile_pool(name="psum", bufs=1, space="PSUM") as psp,
        ):
            # ---- persistent tiles + head DMAs (ordered for earliest start)
            w_sb = {}
            xbox = {}

            def emit_x_load(pname, x_d, sc):
                xt = st.tile(
                    [128, NDM * SQ], bf16, name=f"x{pname}", tag=f"x{pname}", bufs=2
                )
                nc.sync.dma_start(xt[:], x_d[sc])
                xbox[(pname, sc)] = xt

            for pname, w_d, x_d in (
                ("k", wk_d, xk_d),
                ("v", wv_d, xv_d),
                ("q", wq_d, xq_d),
            ):
                wt = res.tile([128, NDM * FH], bf16, name=f"w{pname}", tag=f"w{pname}")
                nc.sync.dma_start(wt[:], w_d[:])
                w_sb[pname] = wt
                emit_x_load(pname, x_d, 0)

            tri = res.tile([128, 128], bf16, name="tri", tag="tri")
            nc.sync.dma_start(tri[:], tri_d[:])
            bq_t = [res.tile([128, 1], f32, name=f"bq{i}", tag=f"bq{i}") for i in range(4)]
            bk_t = [res.tile([128, 1], f32, name=f"bk{i}", tag=f"bk{i}") for i in range(4)]
            for i in range(4):
                nc.sync.dma_start(
                    bq_t[i][:],
                    bq_d[i * 128 : (i + 1) * 128].rearrange("(p o) -> p o", o=1),
                )
                nc.sync.dma_start(
                    bk_t[i][:],
                    bk_d[i * 128 : (i + 1) * 128].rearrange("(p o) -> p o", o=1),
                )
            wo_sb = res.tile([128, 4 * D], bf16, name="wo", tag="wo")
            nc.sync.dma_start(wo_sb[:], wo_d[:])

            kt = [res.tile([128, S], bf16, name=f"kt{i}", tag=f"kt{i}") for i in range(4)]
            # 520 data cols (8 heads x (64 V + ones)) + pad so PV can read a
            # 128-wide stationary slice for head 7 (rows 65+ of the PV output
            # are garbage and ignored)
            vaug = [
                res.tile([128, 584], bf16, name=f"va{k}", tag=f"va{k}")
                for k in range(NKB)
            ]
            # persistent ctx^T tiles: ctx_sb[sb][fc] is [128, 512]
            ctx_sb = [
                [
                    res.tile([128, SQ], bf16, name=f"cx{sb}{fc}", tag=f"cx{sb}{fc}")
                    for fc in range(4)
                ]
                for sb in range(NSC)
            ]
            onesf = res.tile([128, HL], f32, name="onesf", tag="onesf")
            nc.vector.memset(onesf[:], 1.0)
            ones_t = res.tile([128, HL], bf16, name="ones", tag="ones")
            nc.vector.tensor_copy(ones_t[:], onesf[:])
            # write the ones columns of every vaug tile once, up front
            for kb in range(NKB):
                va3 = vaug[kb][:, 0 : HL * 65].rearrange("p (h e) -> p h e", e=65)
                nc.vector.tensor_copy(
                    va3[:, :, 64:65], ones_t[:].rearrange("p (h o) -> p h o", o=1)
                )

            qsf_box = {}  # sc -> [4 q-slice tiles]

            def make_proj_thunk(pname, sc, gi):
                def group(pname=pname, sc=sc, gi=gi):
                    if pname == "q" and gi == 0:
                        qsf_box[sc] = [
                            st.tile(
                                [128, SQ], bf16, name=f"qs{i}", tag=f"qs{i}", bufs=2
                            )
                            for i in range(4)
                        ]
                    xt = xbox[(pname, sc)]
                    w = w_sb[pname]
                    if pname in ("q", "k"):
                        pp = psp.tile([128, SQ], f32, name="pp", tag="pp", bufs=2)
                        for dm in range(NDM):
                            c0 = dm * FH + gi * 128
                            nc.tensor.matmul(
                                pp[:],
                                w[:, c0 : c0 + 128],
                                xt[:, dm * SQ : (dm + 1) * SQ],
                                start=(dm == 0),
                                stop=(dm == NDM - 1),
                            )
                        if pname == "k":
                            nc.vector.tensor_scalar_add(
                                kt[gi][:, sc * SQ : (sc + 1) * SQ], pp[:], bk_t[gi][:]
                            )
                        else:
                            nc.vector.tensor_scalar_add(
                                qsf_box[sc][gi][:], pp[:], bq_t[gi][:]
                            )
                    else:  # v
                        kb = sc * 4 + gi
                        pp = psp.tile([128, FH], f32, name="pp", tag="pp", bufs=2)
                        for dm in range(NDM):
                            c0 = dm * SQ + gi * 128
                            nc.tensor.matmul(
                                pp[:],
                                xt[:, c0 : c0 + 128],
                                w[:, dm * FH : (dm + 1) * FH],
                                start=(dm == 0),
                                stop=(dm == NDM - 1),
                            )
                        va3 = vaug[kb][:, 0 : HL * 65].rearrange(
                            "p (h e) -> p h e", e=65
                        )
                        pp3 = pp[:].rearrange("p (h e) -> p h e", e=64)
                        nc.vector.tensor_copy(va3[:, :, 0:64], pp3[:])

                return group

            def make_proj_thunks(sc, order=None):
                if order is None:
                    order = [("k", g) for g in range(4)] + [("v", g) for g in range(4)] + [
                        ("q", g) for g in range(4)
                    ]
                thunks = []
                loads_done = set()
                for pname, gi in order:
                    if pname not in loads_done and sc > 0:
                        # x load for this (pname, sc) rides with its first group
                        loads_done.add(pname)
                        xd = {"k": xk_d, "v": xv_d, "q": xq_d}[pname]

                        def lg(pname=pname, xd=xd, sc=sc, gi=gi):
                            emit_x_load(pname, xd, sc)
                            make_proj_thunk(pname, sc, gi)()

                        thunks.append(lg)
                    else:
                        thunks.append(make_proj_thunk(pname, sc, gi))
                return thunks

            def make_attn_pairs(h, sb):
                """(score_thunk, pv_thunk) pairs for one (head, superblock).

                kb order: diagonal blocks first [d0..d3] (d0 full query width
                and start=True; d1..d3 trimmed to the causally visible query
                range), then off-diagonal blocks.
                """
                ti, po = h // 2, (h % 2) * 64
                nkb = 4 * (sb + 1)
                kbs = list(range(4 * sb, 4 * sb + 4)) + list(range(4 * sb))
                state = {}

                def score(b0):
                    esl = []
                    for i in range(b0, b0 + 4):
                        kb = kbs[i]
                        dj = kb - sb * 4  # diagonal index (0..3) or negative
                        q0 = dj * 128 if 0 <= dj else 0
                        sp = psp.tile([128, SQ], f32, name="sp", tag="sp", bufs=4)
                        nc.tensor.matmul(
                            sp[:, q0:SQ],
                            kt[ti][po : po + 64, kb * 128 : (kb + 1) * 128],
                            qsf_box[sb][ti][po : po + 64, q0:SQ],
                            start=True,
                            stop=True,
                        )
                        es = st.tile([128, SQ], bf16, name="es", tag="es", bufs=9)
                        nc.scalar.activation(es[:, q0:SQ], sp[:, q0:SQ], AF.Exp)
                        if 0 <= dj:
                            nc.vector.tensor_mul(
                                es[:, q0 : q0 + 128], es[:, q0 : q0 + 128], tri[:]
                            )
                        esl.append((kb, q0, es))
                    state[b0] = esl

                def pv(b0):
                    if b0 == 0:
                        state["cp"] = psp.tile(
                            [128, SQ], f32, name="cp", tag="cp", bufs=2
                        )
                        state["emitted"] = 0
                    cp = state["cp"]
                    for kb, q0, es in state.pop(b0):
                        nc.tensor.matmul(
                            cp[:, q0:SQ],
                            vaug[kb][:, h * 65 : h * 65 + 128],
                            es[:, q0:SQ],
                            start=(state["emitted"] == 0),
                            stop=(state["emitted"] == nkb - 1),
                        )
                        state["emitted"] += 1
                    if b0 + 4 >= nkb:
                        # normalize and write ctx^T slice into SBUF ctx tiles
                        fc, prow = h // 2, (h % 2) * 64
                        d1 = st.tile([1, SQ], f32, name="d1", tag="d1", bufs=2)
                        nc.vector.tensor_copy(d1[:], cp[64:65, :])
                        rb = st.tile([64, SQ], f32, name="rb", tag="rb", bufs=2)
                        nc.gpsimd.partition_broadcast(rb[:], d1[:])
                        rc = st.tile([64, SQ], f32, name="rc", tag="rc", bufs=2)
                        nc.vector.reciprocal_approx_fast(rc[:], rb[:])
                        if prow == 0:
                            nc.vector.tensor_mul(
                                ctx_sb[sb][fc][0:64, :], cp[0:64, :], rc[:]
                            )
                        else:
                            nrm = st.tile(
                                [64, SQ], bf16, name="nrm", tag="nrm", bufs=2
                            )
                            nc.vector.tensor_mul(nrm[:], cp[0:64, :], rc[:])
                            # partition shift 0-63 -> 64-127 via SBUF->SBUF DMA
                            nc.sync.dma_start(ctx_sb[sb][fc][64:128, :], nrm[:])

                return [
                    (
                        (lambda b0=b0: score(b0)),
                        (lambda b0=b0: pv(b0)),
                    )
                    for b0 in range(0, nkb, 4)
                ]

            def make_o_thunks(sb):
                thunks = []
                for qb in range(4):
                    for n2 in range(2):

                        def group(qb=qb, n2=n2, sb=sb):
                            pp = psp.tile([128, SQ], f32, name="pp", tag="pp", bufs=2)
                            for fc in range(4):
                                nc.tensor.matmul(
                                    pp[:],
                                    ctx_sb[sb][fc][:, qb * 128 : (qb + 1) * 128],
                                    wo_sb[:, fc * D + n2 * SQ : fc * D + (n2 + 1) * SQ],
                                    start=(fc == 0),
                                    stop=(fc == 3),
                                )
                            ob = st.tile([128, SQ], bf16, name="ob", tag="ob", bufs=2)
                            nc.vector.tensor_copy(ob[:], pp[:])
                            nc.sync.dma_start(
                                out_d[
                                    sb * SQ + qb * 128 : sb * SQ + (qb + 1) * 128,
                                    n2 * SQ : (n2 + 1) * SQ,
                                ],
                                ob[:],
                            )

                        thunks.append(group)
                return thunks

            def pipeline_stream(pairs):
                # flatten: S_0, S_1, P_0, S_2, P_1, ..., S_n-1, P_n-2, P_n-1
                stream = []
                prev_p = None
                for s_t, p_t in pairs:
                    stream.append(s_t)
                    if prev_p is not None:
                        stream.append(prev_p)
                    prev_p = p_t
                stream.append(prev_p)
                return stream

            def emit_stream(stream, warm):
                nb, nw = len(stream), len(warm)
                wi = 0
                for bi, bt in enumerate(stream):
                    bt()
                    while wi < nw and (wi + 1) * nb <= (bi + 1) * nw:
                        warm[wi]()
                        wi += 1
                while wi < nw:
                    warm[wi]()
                    wi += 1

            # ---- emission schedule ----
            # sb0: minimal prefix of proj(0) [k0 v0..v3 q0], then attention
            # heads interleaved explicitly with the remaining k/q groups
            # (head pair ti becomes eligible right after k_ti/q_ti), with
            # proj(1) woven across the whole superblock.
            p0 = {
                (pn, g): make_proj_thunk(pn, 0, g)
                for pn in ("k", "v", "q")
                for g in range(4)
            }
            for t in (
                p0[("k", 0)],
                p0[("v", 0)],
                p0[("v", 1)],
                p0[("v", 2)],
                p0[("v", 3)],
                p0[("q", 0)],
            ):
                t()
            sp0 = [make_attn_pairs(h, 0)[0] for h in range(HL)]
            s0 = [p[0] for p in sp0]
            v0 = [p[1] for p in sp0]
            stream0 = [
                s0[0], s0[1], v0[0],
                p0[("k", 1)], p0[("q", 1)],
                s0[2], v0[1], s0[3], v0[2],
                p0[("k", 2)], p0[("q", 2)],
                s0[4], v0[3], s0[5], v0[4],
                p0[("k", 3)], p0[("q", 3)],
                s0[6], v0[5], s0[7], v0[6], v0[7],
            ]
            emit_stream(stream0, make_proj_thunks(1))
            for sb in (1, 2, 3):
                pairs = []
                for h in range(HL):
                    pairs += make_attn_pairs(h, sb)
                warm = {
                    1: lambda: make_proj_thunks(2),
                    2: lambda: make_proj_thunks(3) + make_o_thunks(0),
                    3: lambda: make_o_thunks(1) + make_o_thunks(2),
                }[sb]()
                emit_stream(pipeline_stream(pairs), warm)
            for t in make_o_thunks(3):
                t()

    nc.compile()
    return nc


def kernel(
    q,
    k,
    v,
    mask=None,
    Wq=None,
    bq=None,
    Wk=None,
    bk=None,
    Wv=None,
    bv=None,
    Wo=None,
    bo=None,
    **_unused,
):
    import ml_dtypes
    from concourse.bass_utils import run_bass_kernel_spmd

    if "nc" not in _cache:
        _cache["nc"] = _build()
    nc = _cache["nc"]

    bf = ml_dtypes.bfloat16
    q = np.asarray(q, np.float32)
    k = np.asarray(k, np.float32)
    v = np.asarray(v, np.float32)
    Wq = np.asarray(Wq, np.float32)
    Wk = np.asarray(Wk, np.float32)
    Wv = np.asarray(Wv, np.float32)
    Wo = np.asarray(Wo, np.float32)
    bq = np.zeros(D, np.float32) if bq is None else np.asarray(bq, np.float32)
    bk = np.zeros(D, np.float32) if bk is None else np.asarray(bk, np.float32)
    bv = np.zeros(D, np.float32) if bv is None else np.asarray(bv, np.float32)
    bo = np.zeros(D, np.float32) if bo is None else np.asarray(bo, np.float32)

    # triangular causal mask [128, 128]: tri[kk, c] = 1 iff kk <= c
    kk = np.arange(128)[:, None]
    cc = np.arange(128)[None, :]
    tri = (kk <= cc).astype(bf)

    def pack_x(x):  # [S, D] -> [NSC, 128, NDM*SQ]  (sc, p, dm*sq)
        xT = np.ascontiguousarray(x.T)  # [D, S]
        return np.ascontiguousarray(
            xT.reshape(NDM, 128, NSC, SQ).transpose(2, 1, 0, 3).reshape(
                NSC, 128, NDM * SQ
            )
        ).astype(bf)

    def pack_w(wT):  # [D, FH] -> [128, NDM*FH]  (p, dm*fh)
        return np.ascontiguousarray(
            wT.reshape(NDM, 128, FH).transpose(1, 0, 2).reshape(128, NDM * FH)
        ).astype(bf)

    xT = {}
    for b in range(B):
        xT[("q", b)] = pack_x(q[b])
        xT[("k", b)] = pack_x(k[b])
        xT[("v", b)] = pack_x(v[b])
    wqs, wks, wvs, wos, bqs, bks = {}, {}, {}, {}, {}, {}
    for hg in range(2):
        sl = slice(hg * FH, (hg + 1) * FH)
        wqs[hg] = pack_w(Wq[sl, :].T * np.float32(0.125))
        wks[hg] = pack_w(Wk[sl, :].T)
        wvs[hg] = pack_w(Wv[sl, :].T)
        # wo: [FH, D] -> [128, 4*D]  (p, fc*d)
        woT = Wo[:, sl].T
        wos[hg] = np.ascontiguousarray(
            woT.reshape(4, 128, D).transpose(1, 0, 2).reshape(128, 4 * D)
        ).astype(bf)
        bqs[hg] = np.ascontiguousarray(bq[sl]) * np.float32(0.125)
        bks[hg] = np.ascontiguousarray(bk[sl])

    in_maps = []
    for c in range(NCORES):
        b, hg = c // 2, c % 2
        in_maps.append(
            {
                "xq": xT[("q", b)],
                "xk": xT[("k", b)],
                "xv": xT[("v", b)],
                "wq": wqs[hg],
                "wk": wks[hg],
                "wv": wvs[hg],
                "wo": wos[hg],
                "tri": tri,
                "bq": bqs[hg],
                "bk": bks[hg],
            }
        )

    res = run_bass_kernel_spmd(nc, in_maps, list(range(NCORES)))
    out = np.empty((B, S, D), np.float32)
    for b in range(B):
        out[b] = np.asarray(res.results[2 * b]["out"], np.float32) + np.asarray(
            res.results[2 * b + 1]["out"], np.float32
        )
    const = Wo @ bv + bo  # bv/bo contribution (folds exactly through softmax)
    if np.any(const):
        out += const[None, None, :]
    return out
